# revision 28
# baseline (speedup 1.0000x reference)
"""PointTransformerLayer kernel for 8 Trainium2 NeuronCores (Bass/Tile).

Contract: kernel(**inputs) takes the FULL unsharded inputs (as produced by
setup_inputs()) and returns the FULL (B, N, C) float32 output.

Sharding: the flat (B*N) = 32768 points are split into 8 shards of 4096
points (cores 0-3 handle batch 0, cores 4-7 batch 1).  Every core receives a
replicated neighbor table in DRAM and gathers its neighbors' rows with
indirect DMA; projections are fused AFTER the gather so no k/v tables are
ever materialized.

Table row (512B): [ feat_j (128 bf16) | -Wd1 @ xyz_j (128 bf16) ].

All pair-domain compute runs in transposed [C, pairs] layout (512-pair
tiles) so every matmul streams 512-wide moving operands at 1 cycle/row.
Algebraic folds (host-side):
    qg_i  = (Wg1 Wq) f_i + Wg1(bq - bk + bd2) + bg1        [input, per point]
    a_i   = Wd1 xyz_i + bd1                                [input, per point]
    pe1   = a_i - Wd1 xyz_n          (a-bmm + gathered-(-d) as matmul lhsT)
    h1    = relu(qg_i - (Wg1 Wk) f_n + (Wg1 Wd2) relu(pe1))
    e     = exp(Wg2 h1 + bg2)        (no max-subtraction; logits are O(1))
    u     = Wv f_n + Wd2 relu(pe1)   (bv, bd2 folded into final bias)
    out   = (sum_k e*u) / (sum_k e)
    y     = Wo out + (Wo (bd2 + bv) + bo)
Broadcasts over the K=16 neighbors use a 0/1 selection matrix on the PE
(B_rep) for q/a terms, and PSUM accumulation fuses all additive terms.
"""

import numpy as np

B, N, K, C = 2, 16384, 16, 128
NCORES = 8
SH = (B * N) // NCORES          # 4096 own points per core
PTILE = 32                      # points per pair tile
PAIRS = PTILE * K               # 512 pairs per tile
NTILES = SH // PTILE            # 128 pair tiles per core
TBL_W = 256                     # bf16 cols: feat(128) | -d(128); 512B rows

_CACHE = {}


def _build_nc():
    import concourse.bass as bass
    import concourse.mybir as mybir
    import concourse.tile as tile
    from concourse import bacc

    f32 = mybir.dt.float32
    f32r = mybir.dt.float32r
    bf16 = mybir.dt.bfloat16
    i32 = mybir.dt.int32
    AX = mybir.AxisListType
    OP = mybir.AluOpType
    ACTF = mybir.ActivationFunctionType

    nc = bacc.Bacc("TRN2", target_bir_lowering=False, debug=False,
                   num_devices=NCORES)

    # ---- I/O ----
    tbl = nc.dram_tensor("tbl", [B * N, TBL_W], bf16, kind="ExternalInput")
    # idx16[ch, 32*t + col] = flat neighbor index seq[col*16+ch] of tile t
    # (dma_gather wrap order; only partitions 0-15 carry real indices)
    idx16 = nc.dram_tensor("idx16", [C, 32 * NTILES], mybir.dt.int16,
                           kind="ExternalInput")
    qgt = nc.dram_tensor("qgt", [PTILE, NTILES * C], bf16, kind="ExternalInput")
    at = nc.dram_tensor("at", [PTILE, NTILES * C], bf16, kind="ExternalInput")
    wv = nc.dram_tensor("wv", [C, C], bf16, kind="ExternalInput")     # Wv.T
    wkg = nc.dram_tensor("wkg", [C, C], bf16, kind="ExternalInput")   # -(Wg1 Wk).T
    wd2 = nc.dram_tensor("wd2", [C, C], f32r, kind="ExternalInput")   # Wd2.T
    wa = nc.dram_tensor("wa", [C, C], f32r, kind="ExternalInput")     # (Wg1 Wd2).T
    wg2 = nc.dram_tensor("wg2", [C, C], f32r, kind="ExternalInput")   # Wg2.T
    wo = nc.dram_tensor("wo", [C, C], f32r, kind="ExternalInput")     # Wo.T
    bg2c = nc.dram_tensor("bg2c", [C, 1], f32, kind="ExternalInput")
    coc = nc.dram_tensor("coc", [C, 1], f32, kind="ExternalInput")
    brep = nc.dram_tensor("brep", [PTILE, PAIRS], bf16, kind="ExternalInput")
    ibf = nc.dram_tensor("ibf", [C, C], bf16, kind="ExternalInput")  # identity
    yT = nc.dram_tensor("yT", [C, SH], f32, kind="ExternalOutput")

    def R(ap):  # float32r view for full-rate fp32 matmuls
        return ap.bitcast(f32r)

    with tile.TileContext(nc) as tc:
        with (
            tc.tile_pool(name="const", bufs=1) as cp,
            tc.tile_pool(name="res", bufs=1) as rp,
            tc.tile_pool(name="sb", bufs=6) as sb,
            tc.tile_pool(name="cb", bufs=8) as cb,
            tc.tile_pool(name="ps_pe1", bufs=2, space="PSUM") as ps_pe1,
            tc.tile_pool(name="ps_u", bufs=2, space="PSUM") as ps_u,
            tc.tile_pool(name="ps_pl", bufs=2, space="PSUM") as ps_pl,
            tc.tile_pool(name="ps_h1", bufs=2, space="PSUM") as ps_h1,
        ):
            # ---- constants / residents ----
            w_v = cp.tile([C, C], bf16); nc.sync.dma_start(out=w_v[:], in_=wv[:])
            w_kg = cp.tile([C, C], bf16); nc.sync.dma_start(out=w_kg[:], in_=wkg[:])
            w_d2 = cp.tile([C, C], f32r); nc.sync.dma_start(out=w_d2[:], in_=wd2[:])
            w_a = cp.tile([C, C], f32r); nc.sync.dma_start(out=w_a[:], in_=wa[:])
            w_g2 = cp.tile([C, C], f32r); nc.sync.dma_start(out=w_g2[:], in_=wg2[:])
            w_o = cp.tile([C, C], f32r); nc.sync.dma_start(out=w_o[:], in_=wo[:])
            c_g2 = cp.tile([C, 1], f32); nc.sync.dma_start(out=c_g2[:], in_=bg2c[:])
            c_o = cp.tile([C, 1], f32); nc.sync.dma_start(out=c_o[:], in_=coc[:])
            b_rep = cp.tile([PTILE, PAIRS], bf16); nc.sync.dma_start(out=b_rep[:], in_=brep[:])
            id_bf = cp.tile([C, C], bf16); nc.sync.dma_start(out=id_bf[:], in_=ibf[:])
            idx_res = cp.tile([C, 32 * NTILES], mybir.dt.int16)
            nc.sync.dma_start(out=idx_res[:], in_=idx16[:])
            qgT_res = rp.tile([PTILE, NTILES * C], bf16)
            nc.sync.dma_start(out=qgT_res[:], in_=qgt[:])
            aT_res = rp.tile([PTILE, NTILES * C], bf16)
            nc.sync.dma_start(out=aT_res[:], in_=at[:])
            yT_res = rp.tile([C, SH], f32r)

            # ---- pair tiles, 4-stage software pipeline ----
            def emit_gather(t):
                # gather 512 neighbor rows, transposed on the fly:
                # comb[c, 0, i] = feat_n[i][c],  comb[c, 1, i] = -d_n[i][c]
                comb = cb.tile([C, 2 * PAIRS], bf16, tag="comb")
                nc.gpsimd.dma_gather(
                    out_ap=comb[:].rearrange("p (h i) -> p h i", i=PAIRS),
                    in_ap=tbl[:],
                    idxs_ap=idx_res[:, 32 * t:32 * (t + 1)],
                    num_idxs=PAIRS,
                    num_idxs_reg=PAIRS,
                    elem_size=TBL_W,
                    transpose=True)
                return comb

            def emit_f1(t, comb):
                fT_sb = comb[:, 0:PAIRS]          # feat^T, bf16
                dT_sb = comb[:, PAIRS:2 * PAIRS]  # (-Wd1 xyz_n)^T, bf16

                # pe1 = a_i (broadcast over k) - Wd1 xyz_n, all on the PE
                pe1_ps = ps_pe1.tile([C, PAIRS], f32, tag="pe1")
                nc.tensor.matmul(pe1_ps[:], aT_res[:, C * t:C * (t + 1)],
                                 b_rep[:, :], start=True, stop=False)
                nc.tensor.matmul(pe1_ps[:], id_bf[:], dT_sb, start=False, stop=True)
                r1 = sb.tile([C, PAIRS], f32r, tag="r1")
                nc.scalar.activation(out=r1[:], in_=pe1_ps[:], func=ACTF.Relu)
                return fT_sb, r1

            def emit_f2(t, fT_sb, r1):
                u_ps = ps_u.tile([C, PAIRS], f32, tag="u")
                nc.tensor.matmul(u_ps[:], w_d2[:], r1[:], start=True, stop=False)
                nc.tensor.matmul(u_ps[:], w_v[:], fT_sb, start=False, stop=True)

                h1_ps = ps_h1.tile([C, PAIRS], f32, tag="h1")
                nc.tensor.matmul(h1_ps[:], qgT_res[:, C * t:C * (t + 1)],
                                 b_rep[:, :], start=True, stop=False)
                nc.tensor.matmul(h1_ps[:], w_a[:], r1[:], start=False, stop=False)
                nc.tensor.matmul(h1_ps[:], w_kg[:], fT_sb, start=False, stop=True)
                h1_sb = sb.tile([C, PAIRS], f32r, tag="h1sb")
                nc.scalar.activation(out=h1_sb[:], in_=h1_ps[:], func=ACTF.Relu)

                lg_ps = ps_pl.tile([C, PAIRS], f32, tag="pl")
                nc.tensor.matmul(lg_ps[:], w_g2[:], h1_sb[:], start=True, stop=True)
                if t % 2 == 0:
                    e2 = sb.tile([C, 2 * PAIRS], bf16, tag="e")
                else:
                    e2 = None
                etile = e2 if e2 is not None else None
                return lg_ps, u_ps, e2

            def emit_exp(t, lg_ps, e2, half):
                nc.scalar.activation(out=e2[:, PAIRS * half:PAIRS * (half + 1)],
                                     in_=lg_ps[:], func=ACTF.Exp, bias=c_g2[:, :1])

            def emit_tail2(p, e2, u0, u1):
                # tail over a PAIR of tiles (2p, 2p+1): wide reduces on SBUF
                pt0 = 2 * PTILE * p
                w_t = sb.tile([C, 2 * PAIRS], bf16, tag="w")
                nc.vector.tensor_tensor(out=w_t[:, 0:PAIRS],
                                        in0=e2[:, 0:PAIRS], in1=u0[:], op=OP.mult)
                nc.vector.tensor_tensor(out=w_t[:, PAIRS:2 * PAIRS],
                                        in0=e2[:, PAIRS:2 * PAIRS], in1=u1[:], op=OP.mult)
                se = sb.tile([C, 2 * PTILE], f32, tag="se")
                nc.vector.tensor_reduce(out=se[:],
                                        in_=e2[:].rearrange("p (n k) -> p n k", k=K),
                                        axis=AX.X, op=OP.add)
                ws = sb.tile([C, 2 * PTILE], bf16, tag="ws")
                with nc.allow_low_precision("attn-weighted sum, 16 values"):
                    nc.vector.tensor_reduce(out=ws[:],
                                            in_=w_t[:].rearrange("p (n k) -> p n k", k=K),
                                            axis=AX.X, op=OP.add)
                rse = sb.tile([C, 2 * PTILE], f32, tag="rse")
                nc.vector.reciprocal_approx_fast(out=rse[:], in_=se[:])
                nc.gpsimd.tensor_tensor(out=yT_res[:, pt0:pt0 + 2 * PTILE],
                                        in0=ws[:], in1=rse[:], op=OP.mult)

            GLAG = 4
            combq = {}
            q1 = None
            e2_cur = None
            upair = []
            pend_pair = None
            for t in range(GLAG):
                combq[t] = emit_gather(t)
            for t in range(NTILES + 1):
                if t + GLAG < NTILES:
                    combq[t + GLAG] = emit_gather(t + GLAG)
                cur1 = emit_f1(t, combq.pop(t)) if t < NTILES else None
                if q1 is not None:
                    tp = t - 1
                    lg_ps, u_ps, e2_new = emit_f2(tp, *q1)
                    if e2_new is not None:
                        e2_cur = e2_new
                    emit_exp(tp, lg_ps, e2_cur, tp % 2)
                    upair.append(u_ps)
                    if tp % 2 == 1:
                        if pend_pair is not None:
                            emit_tail2(*pend_pair)
                        pend_pair = ((tp - 1) // 2, e2_cur, upair[0], upair[1])
                        upair = []
                q1 = cur1
            if pend_pair is not None:
                emit_tail2(*pend_pair)

            # ---- output projection ----
            for m in range(SH // 512):
                yo_ps = ps_pl.tile([C, 512], f32, tag="pl")
                nc.tensor.matmul(yo_ps[:], w_o[:],
                                 yT_res[:, 512 * m:512 * (m + 1)], start=True, stop=True)
                yo_sb = sb.tile([C, 512], f32, tag="yo")
                nc.scalar.activation(out=yo_sb[:], in_=yo_ps[:],
                                     func=ACTF.Identity, bias=c_o[:, :1])
                nc.sync.dma_start(out=yT[:, 512 * m:512 * (m + 1)], in_=yo_sb[:])

    nc.compile()
    return nc


def get_nc():
    if "nc" not in _CACHE:
        _CACHE["nc"] = _build_nc()
    return _CACHE["nc"]


def make_in_maps(xyz, features, neighbor_indices,
                 Wq, bq, Wk, bk, Wv, bv,
                 Wd1, bd1, Wd2, bd2,
                 Wg1, bg1, Wg2, bg2,
                 Wo, bo):
    import ml_dtypes
    bf = ml_dtypes.bfloat16

    xyz = np.asarray(xyz, np.float32).reshape(B * N, 3)
    feat = np.asarray(features, np.float32).reshape(B * N, C)
    ni = np.asarray(neighbor_indices)
    flat_idx = (ni.astype(np.int64) +
                (np.arange(B, dtype=np.int64) * N)[:, None, None]
                ).reshape(B * N, K).astype(np.int32)

    G1 = np.asarray(Wg1, np.float32)
    Wd1_ = np.asarray(Wd1, np.float32)
    d = xyz @ Wd1_.T                                  # [BN, C]
    tblv = np.zeros((B * N, TBL_W), bf)
    tblv[:, :C] = feat.astype(bf)
    tblv[:, C:] = (-d).astype(bf)

    qg = feat @ (G1 @ np.asarray(Wq, np.float32)).T + (
        G1 @ (np.asarray(bq, np.float32) - np.asarray(bk, np.float32)
              + np.asarray(bd2, np.float32)) + np.asarray(bg1, np.float32))
    a = xyz @ Wd1_.T + np.asarray(bd1, np.float32)    # [BN, C]

    wvv = np.ascontiguousarray(np.asarray(Wv, np.float32).T).astype(bf)
    wkgv = np.ascontiguousarray(-(G1 @ np.asarray(Wk, np.float32)).T).astype(bf)
    wd2v = np.ascontiguousarray(np.asarray(Wd2, np.float32).T)
    wav = np.ascontiguousarray((G1 @ np.asarray(Wd2, np.float32)).T)
    wg2v = np.ascontiguousarray(np.asarray(Wg2, np.float32).T)
    wov = np.ascontiguousarray(np.asarray(Wo, np.float32).T)
    bg2v = np.asarray(bg2, np.float32).reshape(C, 1)
    cov = (np.asarray(Wo, np.float32) @ (np.asarray(bd2, np.float32)
                                         + np.asarray(bv, np.float32))
           + np.asarray(bo, np.float32)).astype(np.float32).reshape(C, 1)
    brepv = np.kron(np.eye(PTILE, dtype=np.float32),
                    np.ones((1, K), np.float32)).astype(bf)
    ibfv = np.eye(C, dtype=np.float32).astype(bf)

    def tile_layout(x):      # [SH, C] -> [PTILE, NTILES*C]
        return np.ascontiguousarray(
            x.reshape(NTILES, PTILE, C).transpose(1, 0, 2).reshape(PTILE, NTILES * C))

    in_maps = []
    for c in range(NCORES):
        r0 = c * SH
        seq = flat_idx[r0:r0 + SH].reshape(NTILES, PAIRS)     # [t, i]
        idx16 = np.zeros((C, 32 * NTILES), np.int16)
        for t in range(NTILES):
            idx16[:, 32 * t:32 * (t + 1)] = np.tile(seq[t].reshape(32, 16).T, (8, 1))
        in_maps.append({
            "tbl": tblv,
            "idx16": idx16,
            "qgt": tile_layout(qg[r0:r0 + SH].astype(bf)),
            "at": tile_layout(a[r0:r0 + SH].astype(bf)),
            "wv": wvv, "wkg": wkgv, "wd2": wd2v, "wa": wav,
            "wg2": wg2v, "wo": wov, "bg2c": bg2v, "coc": cov,
            "brep": brepv, "ibf": ibfv,
        })
    return in_maps


def kernel(**inputs) -> np.ndarray:
    from concourse.bass_utils import run_bass_kernel_spmd

    nc = get_nc()
    in_maps = make_in_maps(**inputs)
    res = run_bass_kernel_spmd(nc, in_maps, core_ids=list(range(NCORES)))
    out = np.empty((B * N, C), np.float32)
    for c in range(NCORES):
        out[c * SH:(c + 1) * SH] = res.results[c]["yT"].T
    return out.reshape(B, N, C)
